# revision 6
# baseline (speedup 1.0000x reference)
"""Trainium2 Bass kernel for nn_ConditionalDLFactorized19 (moe_routing).

Math (T=1024, B=32, C=1024, K=1024, E=16):
    resp  = softmax(key.reshape(B,K) @ assign_w + assign_b)     # (B, E)
    loss  = 0.01 * std(importance, ddof=1) / mean(importance)   # importance = resp.sum(0)
    W_b   = sum_e resp[b,e] * pw_w1[e].reshape(C, C)            # per-batch (O, I)
    y     = x + einsum('boi,tbi->tbo', W, x)

Distribution over 8 NeuronCores:
  - Routing/loss: tiny, replicated on every core.
  - Mixing (resp @ pw_w1): oi-sharded — core j mixes o-rows [128j, 128j+128)
    of every W_b on the PE via a block-diagonal respT trick (4 column-chunks
    packed into one K=64 matmul), output cast to bf16.
  - AllToAll (per own-batch, 8 ranks) redistributes W so core j holds
    W_b (full O x I, bf16) for its 4 batches.
  - Main matmul (y_b = x_b @ W_b^T): batch-parallel, 4 batches/core, bf16
    operands (x^T and W^T obtained via DMA-xbar transpose), fp32 PSUM
    accumulate, fp32 residual add with the original x.
"""

import numpy as np
import ml_dtypes

import concourse.bass as bass
import concourse.mybir as mybir
import concourse.tile as tile
from concourse import bacc
from concourse import bass_utils
from concourse.bass import ds
from concourse.masks import make_identity

T, B, C, E = 1024, 32, 1024, 16
N_CORES = 8
BPC = B // N_CORES        # batches per core = 4
OPC = C // N_CORES        # o-rows per core = 128
OIPC = OPC * C            # oi elements per core = 131072
F32 = mybir.dt.float32
BF16 = mybir.dt.bfloat16

_CACHE = {}


def _phase0_routing(nc, tc, key_d, aw_d, ab_d, loss_d, mixdiag):
    """Routing softmax + loss on every core; fills mixdiag (block-diag respT)."""
    with (
        tc.tile_pool(name="p0", bufs=1) as p0,
        tc.tile_pool(name="ps0", bufs=1, space="PSUM") as ps0,
    ):
        idt = p0.tile([128, 128], F32)
        make_identity(nc, idt[:])

        k_sb = p0.tile([B, C], F32)
        nc.sync.dma_start(k_sb[:], key_d[:])
        aw_sb = p0.tile([128, 8, E], F32)
        nc.sync.dma_start(aw_sb[:], aw_d[:].rearrange("(c p) e -> p c e", p=128))
        ab_sb = p0.tile([1, E], F32)
        nc.sync.dma_start(ab_sb[:], ab_d[:])
        ones_1_32 = p0.tile([1, B], F32)
        nc.gpsimd.memset(ones_1_32[:], 1.0)
        ones_32_1 = p0.tile([B, 1], F32)
        nc.gpsimd.memset(ones_32_1[:], 1.0)

        # key^T  (C on partitions, 8 chunks of 128)
        kT_ps = ps0.tile([128, 8, B], F32)
        for c in range(8):
            nc.tensor.transpose(kT_ps[:, c, :], k_sb[:, ds(128 * c, 128)],
                                idt[0:B, 0:B])
        kT_sb = p0.tile([128, 8, B], F32)
        nc.vector.tensor_copy(kT_sb[:], kT_ps[:])

        # logits = key @ assign_w + assign_b   (B, E) in PSUM
        lg_ps = ps0.tile([B, E], F32)
        for c in range(8):
            nc.tensor.matmul(lg_ps[:], kT_sb[:, c, :], aw_sb[:, c, :],
                             start=(c == 0), stop=False)
        nc.tensor.matmul(lg_ps[:], ones_1_32[:], ab_sb[:], start=False,
                         stop=True)

        # softmax over E (free dim)
        mx = p0.tile([B, 1], F32)
        nc.vector.reduce_max(mx[:], lg_ps[:], axis=mybir.AxisListType.X)
        mxn = p0.tile([B, 1], F32)
        nc.scalar.mul(mxn[:], mx[:], -1.0)
        ex = p0.tile([B, E], F32)
        nc.scalar.activation(ex[:], lg_ps[:],
                             mybir.ActivationFunctionType.Exp, bias=mxn[:])
        sm = p0.tile([B, 1], F32)
        nc.vector.reduce_sum(sm[:], ex[:], axis=mybir.AxisListType.X)
        rs = p0.tile([B, 1], F32)
        nc.vector.reciprocal(rs[:], sm[:])
        resp = p0.tile([B, E], F32)
        nc.scalar.mul(resp[:], ex[:], rs[:])

        # loss = 0.01 * std(importance, ddof=1) / mean(importance)
        imp_ps = ps0.tile([1, E], F32)
        nc.tensor.matmul(imp_ps[:], ones_32_1[:], resp[:])
        imp = p0.tile([1, E], F32)
        nc.vector.tensor_copy(imp[:], imp_ps[:])
        msum = p0.tile([1, 1], F32)
        nc.vector.reduce_sum(msum[:], imp[:], axis=mybir.AxisListType.X)
        mean = p0.tile([1, 1], F32)
        nc.scalar.mul(mean[:], msum[:], 1.0 / E)
        dcen = p0.tile([1, E], F32)
        nc.vector.tensor_scalar_sub(dcen[:], imp[:], mean[:])
        d2 = p0.tile([1, E], F32)
        nc.scalar.square(d2[:], dcen[:])
        vsum = p0.tile([1, 1], F32)
        nc.vector.reduce_sum(vsum[:], d2[:], axis=mybir.AxisListType.X)
        var = p0.tile([1, 1], F32)
        nc.scalar.mul(var[:], vsum[:], 1.0 / (E - 1))
        std = p0.tile([1, 1], F32)
        nc.scalar.sqrt(std[:], var[:])
        rmean = p0.tile([1, 1], F32)
        nc.vector.reciprocal(rmean[:], mean[:])
        l1 = p0.tile([1, 1], F32)
        nc.vector.tensor_mul(l1[:], std[:], rmean[:])
        lossv = p0.tile([1, 1], F32)
        nc.scalar.mul(lossv[:], l1[:], 0.01)
        nc.sync.dma_start(loss_d[:], lossv[:])

        # block-diagonal resp^T (bf16): lhsT[32g+e, 32g+b] = resp[b, e]
        # (groups live at 32-aligned partition bases; rows 32g+16..32g+31
        #  stay zero so the matching pwt rows are ignored)
        rT_ps = ps0.tile([E, B], F32)
        nc.tensor.transpose(rT_ps[:], resp[:], idt[0:B, 0:B])
        nc.gpsimd.memset(mixdiag[:], 0.0)
        for g in range(4):
            nc.vector.tensor_copy(
                mixdiag[32 * g:32 * g + 16, 32 * g:32 * g + 32], rT_ps[:])


def _phase1_mixing(nc, tc, pw_d, mixdiag, a2a_in, a2a_out):
    """Mix this core's o-slice of every W_b; ship via per-batch AllToAll."""
    with (
        tc.tile_pool(name="pw", bufs=2) as pwp,
        tc.tile_pool(name="mps", bufs=6, space="PSUM") as mps,
        tc.tile_pool(name="msb", bufs=2) as msb,
    ):
        for st in range(4):
            pwt = pwp.tile([128, 16, 512], BF16, tag="pwt")
            # rows 32g+16..32g+31 are dead (mixdiag is zero there) but must
            # stay finite for the matmul -> zero the whole tile first
            nc.gpsimd.memset(pwt[:], 0.0)
            pwsrc = pw_d[:, ds(32768 * st, 32768)].rearrange(
                "e (q g c) -> g e q c", q=16, g=4)
            for g in range(4):
                nc.gpsimd.dma_start(pwt[32 * g:32 * g + 16, :, :], pwsrc[g])
            mst = msb.tile([128, 16, 512], BF16, tag="mst")
            for q in range(16):
                mp = mps.tile([128, 512], F32, tag="mp")
                nc.tensor.matmul(mp[:], mixdiag[:], pwt[:, q, :],
                                 start=True, stop=True)
                if q % 2 == 0:
                    nc.vector.tensor_copy(mst[:, q, :], mp[:])
                else:
                    nc.scalar.copy(mst[:, q, :], mp[:])
            # scatter into per-batch AllToAll inputs.
            # mst partition = 32g + 4r + b ; o' = 32st + 2q + g//2,
            # i = (g%2)*512 + c
            msr = mst[:].rearrange("(g r b) q c -> g b r q c", g=4, r=8, b=4)
            for b in range(BPC):
                for g in range(4):
                    dst = a2a_in[b][:].rearrange(
                        "r (oh q2 ob) (ih c) -> r oh q2 ob ih c",
                        oh=4, q2=16, ob=2, ih=2)[:, st, :, g // 2, g % 2, :]
                    nc.sync.dma_start(dst, msr[g, b])

        for b in range(BPC):
            nc.gpsimd.collective_compute(
                "AllToAll", mybir.AluOpType.bypass,
                replica_groups=[list(range(N_CORES))],
                ins=[a2a_in[b].opt()], outs=[a2a_out[b].opt()])


def _phase2_main(nc, tc, x_d, y_d, a2a_out, xscr):
    """Per-own-batch y_b = x_b @ W_b^T + x_b in bf16/fp32."""
    with (
        tc.tile_pool(name="xf", bufs=2) as xfp,
        tc.tile_pool(name="xb", bufs=2) as xbp,
        tc.tile_pool(name="xTp", bufs=2) as xTp,
        tc.tile_pool(name="wTp", bufs=2) as wTp,
        tc.tile_pool(name="yt", bufs=3) as ytp,
        tc.tile_pool(name="psy", bufs=4, space="PSUM") as psy,
    ):
        for b in range(BPC):
            xf = xfp.tile([128, 8, C], F32, tag="xf")
            nc.sync.dma_start(
                xf[:], x_d[:, b, :].rearrange("(m p) i -> p m i", p=128))
            xb = xbp.tile([128, 8, C], BF16, tag="xb")
            nc.vector.tensor_copy(xb[:], xf[:])
            nc.sync.dma_start(
                xscr[b][:].rearrange("(m p) i -> p m i", p=128), xb[:])

            xT = xTp.tile([128, 8, T], BF16, tag="xT")
            for isl in range(8):
                nc.sync.dma_start(xT[:, isl, :],
                                  xscr[b][:, ds(128 * isl, 128)],
                                  transpose=True)
            wT = wTp.tile([128, 8, C], BF16, tag="wT")
            wsrc = a2a_out[b][:].rearrange("k o i -> (k o) i")
            for isl in range(8):
                nc.sync.dma_start(wT[:, isl, :],
                                  wsrc[:, ds(128 * isl, 128)],
                                  transpose=True)

            for m in range(8):
                psa = psy.tile([128, 512], F32, tag="ps")
                psb = psy.tile([128, 512], F32, tag="ps")
                for k in range(8):
                    lh = xT[:, k, ds(128 * m, 128)]
                    nc.tensor.matmul(psa[:], lh, wT[:, k, 0:512],
                                     start=(k == 0), stop=(k == 7))
                    nc.tensor.matmul(psb[:], lh, wT[:, k, 512:1024],
                                     start=(k == 0), stop=(k == 7))
                yt = ytp.tile([128, C], F32, tag="yt")
                nc.vector.tensor_add(yt[:, 0:512], psa[:], xf[:, m, 0:512])
                nc.vector.tensor_add(yt[:, 512:1024], psb[:],
                                     xf[:, m, 512:1024])
                nc.sync.dma_start(y_d[ds(128 * m, 128), b, :], yt[:])


def _build():
    nc = bacc.Bacc("TRN2", target_bir_lowering=False, debug=False,
                   num_devices=N_CORES)

    x_d = nc.dram_tensor("x", [T, BPC, C], F32, kind="ExternalInput")
    pw_d = nc.dram_tensor("pw", [E, OIPC], F32, kind="ExternalInput")
    key_d = nc.dram_tensor("key", [B, C], F32, kind="ExternalInput")
    aw_d = nc.dram_tensor("aw", [C, E], F32, kind="ExternalInput")
    ab_d = nc.dram_tensor("ab", [1, E], F32, kind="ExternalInput")
    y_d = nc.dram_tensor("y", [T, BPC, C], F32, kind="ExternalOutput")
    loss_d = nc.dram_tensor("loss", [1, 1], F32, kind="ExternalOutput")

    with tile.TileContext(nc) as tc:
        with tc.tile_pool(name="dram", bufs=1, space="DRAM") as dram:
            a2a_in = [dram.tile([N_CORES, OPC, C], BF16, name=f"a2ai{b}")
                      for b in range(BPC)]
            a2a_out = [dram.tile([N_CORES, OPC, C], BF16, name=f"a2ao{b}")
                       for b in range(BPC)]
            xscr = [dram.tile([T, C], BF16, name=f"xscr{b}")
                    for b in range(BPC)]

            with tc.tile_pool(name="pmix", bufs=1) as pmix:
                mixdiag = pmix.tile([128, 128], BF16)
                _phase0_routing(nc, tc, key_d, aw_d, ab_d, loss_d, mixdiag)
                _phase1_mixing(nc, tc, pw_d, mixdiag, a2a_in, a2a_out)

            _phase2_main(nc, tc, x_d, y_d, a2a_out, xscr)

    nc.compile()
    return nc


def _get_nc():
    if "nc" not in _CACHE:
        _CACHE["nc"] = _build()
    return _CACHE["nc"]


def _in_maps(x, key, assign_w, assign_b, pw_w1):
    key2 = np.ascontiguousarray(key.reshape(B, C))
    ab = np.ascontiguousarray(assign_b.reshape(1, E))
    aw = np.ascontiguousarray(assign_w)
    maps = []
    for c in range(N_CORES):
        maps.append({
            "x": np.ascontiguousarray(x[:, c * BPC:(c + 1) * BPC, :]),
            "pw": np.ascontiguousarray(pw_w1[:, c * OIPC:(c + 1) * OIPC]),
            "key": key2,
            "aw": aw,
            "ab": ab,
        })
    return maps


def kernel(x, key, assign_w, assign_b, pw_w1):
    x = np.asarray(x, dtype=np.float32)
    key = np.asarray(key, dtype=np.float32)
    assign_w = np.asarray(assign_w, dtype=np.float32)
    assign_b = np.asarray(assign_b, dtype=np.float32)
    pw_w1 = np.asarray(pw_w1, dtype=np.float32)

    nc = _get_nc()
    res = bass_utils.run_bass_kernel_spmd(nc, _in_maps(x, key, assign_w,
                                                       assign_b, pw_w1),
                                          core_ids=list(range(N_CORES)))
    y = np.concatenate([res.results[c]["y"] for c in range(N_CORES)], axis=1)
    loss = np.float32(res.results[0]["loss"].reshape(())[()])
    return y, loss


def profile_once(inputs):
    """Run once with NTFF tracing; return max-core exec time in ns (or None)."""
    nc = _get_nc()
    res = bass_utils.run_bass_kernel_spmd(
        nc, _in_maps(**inputs), core_ids=list(range(N_CORES)), trace=True)
    return res.exec_time_ns
